# revision 16
# baseline (speedup 1.0000x reference)
"""Pointer-attention kernel for Trainium2 (8 NeuronCores, data-parallel over batch).

Computes, for P = pointer_input [B, S, R], weights W1/W2 [2R]:
    scores = P @ W1[:R] + (h @ W1[R:])[:, None]      # h-term is constant over S
    a      = softmax(scores, axis=S)                 #   -> cancels in softmax
    c      = einsum('bsr,bs->br', P, a)
    pi     = P @ W2[:R] + (c @ W2[R:])[:, None]

Math used here (exact):
    s1[b,s]  = P[b,s,:] . w1p          (w1p = W1[:R])
    E        = exp(s1)                 (softmax shift cancels; inputs are O(1))
    Z[b]     = sum_s E[b,s]
    craw[b,:]= sum_s E[b,s] * P[b,s,:]
    g[b]     = (craw[b,:] . w2c) / Z[b]            (w2c = W2[R:])
    pi[b,s]  = P[b,s,:] . w2p + g[b]               (w2p = W2[:R])

so h_t and W1[R:] never affect the output. One single pass over P.

Measured-cost engine split (per 128x512 s-tile; 8 batches x 16 tiles per core):
  - input DMA on HWDGE (nc.sync): 2 MiB p-major super-tiles in two 1 MiB
    halves -> 16 KB contiguous descriptors; GpSimd/Pool does NO compute or
    DMA (measured: concurrent Q7 tensor ops throttle DVE ~7x).
  - craw on TensorE: lhsT = exp column (bf16), rhs = the *high half-word
    view* of the fp32 tile (stride-2 bf16 bitcast) - P is never cast.
  - matvec tasks (s1+pw2 per tile) run on two engines:
      DVE path: fused scalar_tensor_tensor fp32 (~0.68us/task)
      PE path (tile pairs): 8 PE transposes (fp32, via identity) -> ACT
        evict -> 4 accumulating fp32 matmuls with [w1|w2] stationary
        (both scores at once) -> tiny PE back-transpose -> ACT evict into
        the score columns. Offloads 4 tasks/pair to the idle TensorE.
  - scores kept interleaved in sc_b [128, nt, 2] (s1, pw2) so the PE path
    lands both columns with one copy; exp/pi read strided views.
Per-b epilogue: Z via ones-matmul, dq = craw.w2c (fused DVE op), g = dq/Z,
broadcast via ones-matmul, pi = pw2 + g on ScalarE, DMA out on the scalar
HWDGE ring (tiny packets never block the input stream).
"""

import numpy as np

B, S, R = 64, 2048, 512
N_CORES = 8
B_LOC = B // N_CORES          # 8 batches per core
P_PART = 128                  # partitions per s-tile
NT = S // P_PART              # 16 s-tiles per batch

_CACHED_NC = None

# build-time strategy knobs (tuned from traces)
CFG = dict(
    st=8,            # s-tiles per DMA super-tile (8 -> 2 MiB transfers)
    pe_pairs=3,      # of 4 tile-pairs per super-tile, how many via TensorE
    dma_eng="sync",  # sync | gpsimd
)


def _build_nc(cfg=None, b_loc=B_LOC, nt=NT, finalize=True):
    import concourse.bacc as bacc
    import concourse.bass as bass
    import concourse.mybir as mybir
    import concourse.tile as tile

    cfg = dict(CFG, **(cfg or {}))
    f32 = mybir.dt.float32
    bf16 = mybir.dt.bfloat16
    st_sz = cfg["st"]
    s_loc = nt * P_PART
    assert nt % st_sz == 0
    nst = nt // st_sz
    n_pairs_st = st_sz // 2            # tile-pairs per super-tile
    nc = bacc.Bacc(None, target_bir_lowering=False, debug=True)

    p_h = nc.declare_dram_parameter("p", [b_loc, s_loc, R], f32, isOutput=False)
    w1_h = nc.declare_dram_parameter("w1", [2 * R], f32, isOutput=False)
    w2_h = nc.declare_dram_parameter("w2", [2 * R], f32, isOutput=False)
    id_h = nc.declare_dram_parameter("ident", [P_PART, P_PART], f32, isOutput=False)
    out_h = nc.declare_dram_parameter("out", [b_loc, s_loc], f32, isOutput=True)

    def bcast_ap(src_ap, parts):
        # replicate a 1-D DRAM slice across `parts` partitions
        return bass.AP(
            tensor=src_ap.tensor,
            offset=src_ap.offset,
            ap=[[0, parts]] + [list(d) for d in src_ap.ap],
        )

    with tile.TileContext(nc) as tc:
        dma_eng = nc.sync if cfg["dma_eng"] == "sync" else nc.gpsimd
        with (
            tc.tile_pool(name="consts", bufs=1) as consts,
            tc.tile_pool(name="ptiles", bufs=5) as ptiles,
            tc.tile_pool(name="scratch", bufs=6) as scratch,
            tc.tile_pool(name="ptsb", bufs=3) as ptsb,
            tc.tile_pool(name="perb", bufs=3) as perb,
            tc.tile_pool(name="smalls", bufs=4) as smalls,
            tc.tile_pool(name="psum_c", bufs=1, space="PSUM") as psum_c,
            tc.tile_pool(name="psum_t", bufs=3, space="PSUM") as psum_t,
            tc.tile_pool(name="psum_m", bufs=2, space="PSUM") as psum_m,
            tc.tile_pool(name="psum_b", bufs=1, space="PSUM") as psum_b,
            tc.tile_pool(name="psum_s", bufs=1, space="PSUM") as psum_s,
        ):
            # ---- constants (all tiny row loads + on-chip broadcast) ----
            ident = consts.tile([P_PART, P_PART], f32)
            nc.sync.dma_start(out=ident[:], in_=id_h[:, :])
            w1row = consts.tile([1, R], f32)
            nc.sync.dma_start(out=w1row[:], in_=bcast_ap(w1_h[0:R], 1))
            w2row = consts.tile([1, R], f32)
            nc.sync.dma_start(out=w2row[:], in_=bcast_ap(w2_h[0:R], 1))
            w2c = consts.tile([1, R], f32)
            nc.sync.dma_start(out=w2c[:], in_=bcast_ap(w2_h[R : 2 * R], 1))
            # w12t[rl, rc*2+k]: weight value w_k[rc*128+rl] down the
            # partitions (stationary for the PE matvec; one-time load)
            w12t = consts.tile([P_PART, 8], f32)
            nc.gpsimd.dma_start(
                out=w12t[:, 0::2],
                in_=w1_h[0:R].rearrange("(rc rl) -> rl rc", rl=P_PART),
            )
            nc.gpsimd.dma_start(
                out=w12t[:, 1::2],
                in_=w2_h[0:R].rearrange("(rc rl) -> rl rc", rl=P_PART),
            )
            ones_col = consts.tile([P_PART, 1], f32)
            nc.vector.memset(ones_col[:], 1.0)
            ones_row = consts.tile([1, P_PART], f32)
            nc.vector.memset(ones_row[:], 1.0)
            # partition-broadcast weights for the DVE stt path
            w1p = consts.tile([P_PART, R], f32)
            w2p = consts.tile([P_PART, R], f32)
            for wrow, wp in ((w1row, w1p), (w2row, w2p)):
                w_ps = psum_t.tile([P_PART, R], f32, tag="pt_ps")
                nc.tensor.matmul(
                    w_ps[:], lhsT=ones_row[:], rhs=wrow[:], start=True, stop=True
                )
                nc.scalar.copy(out=wp[:], in_=w_ps[:])

            for b in range(b_loc):
                c_ps = psum_c.tile([1, R], f32, tag="c_ps")
                # interleaved scores: sc_b[:, t, 0] = s1, sc_b[:, t, 1] = pw2
                sc_b = perb.tile([P_PART, nt, 2], f32, tag="sc_b")
                e_bf = perb.tile([P_PART, nt], bf16, tag="e_bf")

                for sti in range(nst):
                    pt = ptiles.tile([P_PART, st_sz * R], f32, tag="pt")
                    src = p_h[b, sti * st_sz * P_PART : (sti + 1) * st_sz * P_PART, :]
                    # p-major: partition p holds st_sz consecutive DRAM rows
                    # -> 16 KB contiguous runs; two 1 MiB halves so consumers
                    # of the first half start before the second half lands.
                    src2 = src.rearrange("(p t) r -> p (t r)", p=P_PART)
                    hw = st_sz * R // 2
                    dma_eng.dma_start(out=pt[:, :hw], in_=src2[:, :hw])
                    dma_eng.dma_start(out=pt[:, hw:], in_=src2[:, hw:])

                    for pr in range(n_pairs_st):
                        j0 = 2 * pr
                        if pr < cfg["pe_pairs"]:
                            # ---- PE path: transpose pair, matvec, back ----
                            pt_sb = ptsb.tile([P_PART, 2 * R], f32, tag="pt_sb")
                            for u in range(2):
                                pt_ps = psum_t.tile([P_PART, R], f32, tag="pt_ps")
                                for rc in range(4):
                                    nc.tensor.transpose(
                                        pt_ps[:, rc * P_PART : (rc + 1) * P_PART],
                                        pt[:, (j0 + u) * R + rc * P_PART : (j0 + u) * R + (rc + 1) * P_PART],
                                        ident[:],
                                    )
                                nc.scalar.copy(
                                    out=pt_sb[:, u * R : (u + 1) * R], in_=pt_ps[:]
                                )
                            ptv = pt_sb[:].rearrange(
                                "p (u rc s) -> p rc u s", u=2, rc=4
                            )
                            mv_ps = psum_m.tile([2, 2 * P_PART], f32, tag="mv_ps")
                            for rc in range(4):
                                nc.tensor.matmul(
                                    mv_ps[:],
                                    lhsT=w12t[:, rc * 2 : rc * 2 + 2],
                                    rhs=ptv[:, rc, :, :],
                                    start=(rc == 0),
                                    stop=(rc == 3),
                                )
                            mv_sb = smalls.tile([2, 2 * P_PART], f32, tag="mv_sb")
                            nc.scalar.copy(out=mv_sb[:], in_=mv_ps[:])
                            for u in range(2):
                                t = sti * st_sz + j0 + u
                                bt_ps = psum_b.tile([P_PART, 2], f32, tag="bt_ps")
                                nc.tensor.transpose(
                                    bt_ps[:],
                                    mv_sb[:, u * P_PART : (u + 1) * P_PART],
                                    ident[0:2, 0:2],
                                )
                                nc.scalar.copy(
                                    out=sc_b[:, t, :], in_=bt_ps[:]
                                )
                        else:
                            # ---- DVE path: fused stt per tile & weight ----
                            for u in range(2):
                                t = sti * st_sz + j0 + u
                                for w, wp in ((0, w1p), (1, w2p)):
                                    prod = scratch.tile([P_PART, R], f32, tag="prod")
                                    nc.vector.scalar_tensor_tensor(
                                        out=prod[:],
                                        in0=pt[:, (j0 + u) * R : (j0 + u + 1) * R],
                                        scalar=1.0,
                                        in1=wp[:],
                                        op0=mybir.AluOpType.mult,
                                        op1=mybir.AluOpType.mult,
                                        accum_out=sc_b[:, t, w : w + 1],
                                    )

                    # ---- exp of this super-tile's score columns (bf16) ----
                    nc.scalar.activation(
                        out=e_bf[:, sti * st_sz : (sti + 1) * st_sz],
                        in_=sc_b[:, sti * st_sz : (sti + 1) * st_sz, 0],
                        func=mybir.ActivationFunctionType.Exp,
                    )
                    # ---- craw accumulation on TensorE (hi-half view) ----
                    for j in range(st_sz):
                        t = sti * st_sz + j
                        rhs_hi = pt[:, j * R : (j + 1) * R].bitcast(bf16)[:, 1::2]
                        nc.tensor.matmul(
                            c_ps[:],
                            lhsT=e_bf[:, t : t + 1],
                            rhs=rhs_hi,
                            start=(t == 0),
                            stop=(t == nt - 1),
                        )

                # ---- per-batch epilogue (all tiny, fp32) ----
                es = smalls.tile([P_PART, 1], f32, tag="es")
                nc.vector.reduce_sum(es[:], e_bf[:], axis=mybir.AxisListType.X)
                zg_ps = psum_s.tile([P_PART, 2], f32, tag="zg_ps")
                z_ps = zg_ps[0:1, 0:1]
                nc.tensor.matmul(
                    z_ps, lhsT=es[:], rhs=ones_col[:], start=True, stop=True
                )
                c_sb = smalls.tile([1, R], f32, tag="c_sb")
                nc.scalar.copy(out=c_sb[:], in_=c_ps[:])
                zr = smalls.tile([1, 1], f32, tag="zr")
                nc.vector.reciprocal(out=zr[:], in_=z_ps)
                cprod = smalls.tile([1, R], f32, tag="cprod")
                dq = smalls.tile([1, 1], f32, tag="dq")
                nc.vector.scalar_tensor_tensor(
                    out=cprod[:],
                    in0=c_sb[:],
                    scalar=1.0,
                    in1=w2c[:],
                    op0=mybir.AluOpType.mult,
                    op1=mybir.AluOpType.mult,
                    accum_out=dq[:],
                )
                g = smalls.tile([1, 1], f32, tag="g")
                nc.vector.tensor_mul(g[:], dq[:], zr[:])
                g_ps = zg_ps[:, 1:2]
                nc.tensor.matmul(
                    g_ps, lhsT=ones_row[:], rhs=g[:], start=True, stop=True,
                    skip_group_check=True,
                )
                g_bc = smalls.tile([P_PART, 1], f32, tag="g_bc")
                nc.scalar.copy(out=g_bc[:], in_=g_ps)
                pi_b = perb.tile([P_PART, nt], f32, tag="pi_b")
                nc.scalar.activation(
                    out=pi_b[:],
                    in_=sc_b[:, :, 1],
                    func=mybir.ActivationFunctionType.Identity,
                    bias=g_bc[:],
                    scale=1.0,
                )
                # s decomposes as (st, p, j): 16 B DRAM runs; scalar HWDGE
                # ring so the tiny packets never block the input stream.
                nc.scalar.dma_start(
                    out=out_h[b].rearrange(
                        "(st p j) -> p st j", p=P_PART, j=st_sz
                    ),
                    in_=pi_b[:],
                )

    if finalize:
        nc.finalize()
    return nc


def _get_nc():
    global _CACHED_NC
    if _CACHED_NC is None:
        _CACHED_NC = _build_nc()
    return _CACHED_NC


def run_sharded(pointer_input, W1, W2, trace=False, trace_kwargs=None, nc=None):
    """Run the SPMD kernel; returns (full_output [1,B,S], BassKernelResults)."""
    from concourse.bass_utils import run_bass_kernel_spmd

    if nc is None:
        nc = _get_nc()
    pointer_input = np.ascontiguousarray(pointer_input, dtype=np.float32)
    W1 = np.ascontiguousarray(W1, dtype=np.float32)
    W2 = np.ascontiguousarray(W2, dtype=np.float32)
    ident = np.eye(P_PART, dtype=np.float32)
    in_maps = [
        {
            "p": pointer_input[i * B_LOC : (i + 1) * B_LOC],
            "w1": W1,
            "w2": W2,
            "ident": ident,
        }
        for i in range(N_CORES)
    ]
    kw = dict(trace_kwargs or {})
    res = run_bass_kernel_spmd(
        nc, in_maps, list(range(N_CORES)), trace=trace, **kw
    )
    out = np.concatenate([res.results[i]["out"] for i in range(N_CORES)], axis=0)
    return out[None].astype(np.float32), res


def kernel(pointer_input, h_t, W1, W2):
    # h_t only shifts scores by a per-batch constant, which softmax cancels;
    # it does not affect the output.
    out, _ = run_sharded(pointer_input, W1, W2, trace=False)
    return out


# revision 17
# speedup vs baseline: 1.4999x; 1.4999x over previous
"""Pointer-attention kernel for Trainium2 (8 NeuronCores, data-parallel over batch).

Computes, for P = pointer_input [B, S, R], weights W1/W2 [2R]:
    scores = P @ W1[:R] + (h @ W1[R:])[:, None]      # h-term is constant over S
    a      = softmax(scores, axis=S)                 #   -> cancels in softmax
    c      = einsum('bsr,bs->br', P, a)
    pi     = P @ W2[:R] + (c @ W2[R:])[:, None]

Math used here (exact):
    s1[b,s]  = P[b,s,:] . w1p          (w1p = W1[:R])
    E        = exp(s1)                 (softmax shift cancels; inputs are O(1))
    Z[b]     = sum_s E[b,s]
    craw[b,:]= sum_s E[b,s] * P[b,s,:]
    g[b]     = (craw[b,:] . w2c) / Z[b]            (w2c = W2[R:])
    pi[b,s]  = P[b,s,:] . w2p + g[b]               (w2p = W2[:R])

so h_t and W1[R:] never affect the output. One single pass over P.

Measured-cost engine split (per 128x512 s-tile; 8 batches x 16 tiles per core):
  - input DMA on HWDGE (nc.sync): 2 MiB p-major super-tiles in two 1 MiB
    halves -> 16 KB contiguous descriptors; GpSimd/Pool does NO compute or
    DMA (measured: concurrent Q7 tensor ops throttle DVE ~7x).
  - craw on TensorE: lhsT = exp column (bf16), rhs = the *high half-word
    view* of the fp32 tile (stride-2 bf16 bitcast) - P is never cast.
  - matvec tasks (s1+pw2 per tile) run on two engines:
      DVE path: fused scalar_tensor_tensor fp32 (~0.68us/task)
      PE path (tile pairs): 8 PE transposes (fp32, via identity) -> ACT
        evict -> 4 accumulating fp32 matmuls with [w1|w2] stationary
        (both scores at once) -> tiny PE back-transpose -> ACT evict into
        the score columns. Offloads 4 tasks/pair to the idle TensorE.
  - scores kept interleaved in sc_b [128, nt, 2] (s1, pw2) so the PE path
    lands both columns with one copy; exp/pi read strided views.
Per-b epilogue: Z via ones-matmul, dq = craw.w2c (fused DVE op), g = dq/Z,
broadcast via ones-matmul, pi = pw2 + g on ScalarE, DMA out on the scalar
HWDGE ring (tiny packets never block the input stream).
"""

import numpy as np

B, S, R = 64, 2048, 512
N_CORES = 8
B_LOC = B // N_CORES          # 8 batches per core
P_PART = 128                  # partitions per s-tile
NT = S // P_PART              # 16 s-tiles per batch

_CACHED_NC = None

# build-time strategy knobs (tuned from traces)
CFG = dict(
    st=8,            # s-tiles per DMA super-tile (8 -> 2 MiB transfers)
    pe_pairs=23,     # of 64 tile-pairs per core, how many via TensorE
    dma_eng="sync",  # sync | gpsimd
)


def _build_nc(cfg=None, b_loc=B_LOC, nt=NT, finalize=True):
    import concourse.bacc as bacc
    import concourse.bass as bass
    import concourse.mybir as mybir
    import concourse.tile as tile

    cfg = dict(CFG, **(cfg or {}))
    f32 = mybir.dt.float32
    bf16 = mybir.dt.bfloat16
    st_sz = cfg["st"]
    s_loc = nt * P_PART
    assert nt % st_sz == 0
    nst = nt // st_sz
    n_pairs_st = st_sz // 2            # tile-pairs per super-tile
    nc = bacc.Bacc(None, target_bir_lowering=False, debug=True)

    p_h = nc.declare_dram_parameter("p", [b_loc, s_loc, R], f32, isOutput=False)
    w1_h = nc.declare_dram_parameter("w1", [2 * R], f32, isOutput=False)
    w2_h = nc.declare_dram_parameter("w2", [2 * R], f32, isOutput=False)
    id_h = nc.declare_dram_parameter("ident", [P_PART, P_PART], f32, isOutput=False)
    out_h = nc.declare_dram_parameter("out", [b_loc, s_loc], f32, isOutput=True)

    def bcast_ap(src_ap, parts):
        # replicate a 1-D DRAM slice across `parts` partitions
        return bass.AP(
            tensor=src_ap.tensor,
            offset=src_ap.offset,
            ap=[[0, parts]] + [list(d) for d in src_ap.ap],
        )

    with tile.TileContext(nc) as tc:
        dma_eng = nc.sync if cfg["dma_eng"] == "sync" else nc.gpsimd
        with (
            tc.tile_pool(name="consts", bufs=1) as consts,
            tc.tile_pool(name="ptiles", bufs=5) as ptiles,
            tc.tile_pool(name="scratch", bufs=6) as scratch,
            tc.tile_pool(name="ptsb", bufs=3) as ptsb,
            tc.tile_pool(name="perb", bufs=3) as perb,
            tc.tile_pool(name="smalls", bufs=4) as smalls,
            tc.tile_pool(name="psum_c", bufs=1, space="PSUM") as psum_c,
            tc.tile_pool(name="psum_t", bufs=3, space="PSUM") as psum_t,
            tc.tile_pool(name="psum_m", bufs=2, space="PSUM") as psum_m,
            tc.tile_pool(name="psum_b", bufs=1, space="PSUM") as psum_b,
            tc.tile_pool(name="psum_s", bufs=1, space="PSUM") as psum_s,
        ):
            # ---- constants (all tiny row loads + on-chip broadcast) ----
            ident = consts.tile([P_PART, P_PART], f32)
            nc.sync.dma_start(out=ident[:], in_=id_h[:, :])
            w1row = consts.tile([1, R], f32)
            nc.sync.dma_start(out=w1row[:], in_=bcast_ap(w1_h[0:R], 1))
            w2row = consts.tile([1, R], f32)
            nc.sync.dma_start(out=w2row[:], in_=bcast_ap(w2_h[0:R], 1))
            w2c = consts.tile([1, R], f32)
            nc.sync.dma_start(out=w2c[:], in_=bcast_ap(w2_h[R : 2 * R], 1))
            # w12t[rl, rc*2+k]: weight value w_k[rc*128+rl] down the
            # partitions (stationary for the PE matvec; one-time load)
            w12t = consts.tile([P_PART, 8], f32)
            nc.gpsimd.dma_start(
                out=w12t[:, 0::2],
                in_=w1_h[0:R].rearrange("(rc rl) -> rl rc", rl=P_PART),
            )
            nc.gpsimd.dma_start(
                out=w12t[:, 1::2],
                in_=w2_h[0:R].rearrange("(rc rl) -> rl rc", rl=P_PART),
            )
            ones_col = consts.tile([P_PART, 1], f32)
            nc.vector.memset(ones_col[:], 1.0)
            ones_row = consts.tile([1, P_PART], f32)
            nc.vector.memset(ones_row[:], 1.0)
            # partition-broadcast weights for the DVE stt path
            w1p = consts.tile([P_PART, R], f32)
            w2p = consts.tile([P_PART, R], f32)
            for wrow, wp in ((w1row, w1p), (w2row, w2p)):
                w_ps = psum_t.tile([P_PART, R], f32, tag="pt_ps")
                nc.tensor.matmul(
                    w_ps[:], lhsT=ones_row[:], rhs=wrow[:], start=True, stop=True
                )
                nc.scalar.copy(out=wp[:], in_=w_ps[:])

            for b in range(b_loc):
                c_ps = psum_c.tile([1, R], f32, tag="c_ps")
                # interleaved scores: sc_b[:, t, 0] = s1, sc_b[:, t, 1] = pw2
                sc_b = perb.tile([P_PART, nt, 2], f32, tag="sc_b")
                e_bf = perb.tile([P_PART, nt], bf16, tag="e_bf")

                for sti in range(nst):
                    pt = ptiles.tile([P_PART, st_sz * R], f32, tag="pt")
                    src = p_h[b, sti * st_sz * P_PART : (sti + 1) * st_sz * P_PART, :]
                    # p-major: partition p holds st_sz consecutive DRAM rows
                    # -> 16 KB contiguous runs; two 1 MiB halves so consumers
                    # of the first half start before the second half lands.
                    src2 = src.rearrange("(p t) r -> p (t r)", p=P_PART)
                    hw = st_sz * R // 2
                    dma_eng.dma_start(out=pt[:, :hw], in_=src2[:, :hw])
                    dma_eng.dma_start(out=pt[:, hw:], in_=src2[:, hw:])

                    for pr in range(n_pairs_st):
                        j0 = 2 * pr
                        gp = (b * nst + sti) * n_pairs_st + pr
                        n_all = b_loc * nst * n_pairs_st
                        if (gp * cfg["pe_pairs"]) % n_all < cfg["pe_pairs"]:
                            # ---- PE path: transpose pair, matvec, back ----
                            pt_sb = ptsb.tile([P_PART, 2 * R], f32, tag="pt_sb")
                            for u in range(2):
                                pt_ps = psum_t.tile([P_PART, R], f32, tag="pt_ps")
                                for rc in range(4):
                                    nc.tensor.transpose(
                                        pt_ps[:, rc * P_PART : (rc + 1) * P_PART],
                                        pt[:, (j0 + u) * R + rc * P_PART : (j0 + u) * R + (rc + 1) * P_PART],
                                        ident[:],
                                    )
                                nc.scalar.copy(
                                    out=pt_sb[:, u * R : (u + 1) * R], in_=pt_ps[:]
                                )
                            ptv = pt_sb[:].rearrange(
                                "p (u rc s) -> p rc u s", u=2, rc=4
                            )
                            mv_ps = psum_m.tile([2, 2 * P_PART], f32, tag="mv_ps")
                            for rc in range(4):
                                nc.tensor.matmul(
                                    mv_ps[:],
                                    lhsT=w12t[:, rc * 2 : rc * 2 + 2],
                                    rhs=ptv[:, rc, :, :],
                                    start=(rc == 0),
                                    stop=(rc == 3),
                                )
                            mv_sb = smalls.tile([2, 2 * P_PART], f32, tag="mv_sb")
                            nc.scalar.copy(out=mv_sb[:], in_=mv_ps[:])
                            for u in range(2):
                                t = sti * st_sz + j0 + u
                                bt_ps = psum_b.tile([P_PART, 2], f32, tag="bt_ps")
                                nc.tensor.transpose(
                                    bt_ps[:],
                                    mv_sb[:, u * P_PART : (u + 1) * P_PART],
                                    ident[0:2, 0:2],
                                )
                                nc.scalar.copy(
                                    out=sc_b[:, t, :], in_=bt_ps[:]
                                )
                        else:
                            # ---- DVE path: fused stt per tile & weight ----
                            for u in range(2):
                                t = sti * st_sz + j0 + u
                                for w, wp in ((0, w1p), (1, w2p)):
                                    prod = scratch.tile([P_PART, R], f32, tag="prod")
                                    nc.vector.scalar_tensor_tensor(
                                        out=prod[:],
                                        in0=pt[:, (j0 + u) * R : (j0 + u + 1) * R],
                                        scalar=1.0,
                                        in1=wp[:],
                                        op0=mybir.AluOpType.mult,
                                        op1=mybir.AluOpType.mult,
                                        accum_out=sc_b[:, t, w : w + 1],
                                    )

                    # ---- exp of this super-tile's score columns (bf16) ----
                    nc.scalar.activation(
                        out=e_bf[:, sti * st_sz : (sti + 1) * st_sz],
                        in_=sc_b[:, sti * st_sz : (sti + 1) * st_sz, 0],
                        func=mybir.ActivationFunctionType.Exp,
                    )
                    # ---- craw accumulation on TensorE (hi-half view) ----
                    for j in range(st_sz):
                        t = sti * st_sz + j
                        rhs_hi = pt[:, j * R : (j + 1) * R].bitcast(bf16)[:, 1::2]
                        nc.tensor.matmul(
                            c_ps[:],
                            lhsT=e_bf[:, t : t + 1],
                            rhs=rhs_hi,
                            start=(t == 0),
                            stop=(t == nt - 1),
                        )

                # ---- per-batch epilogue (all tiny, fp32) ----
                es = smalls.tile([P_PART, 1], f32, tag="es")
                nc.vector.reduce_sum(es[:], e_bf[:], axis=mybir.AxisListType.X)
                zg_ps = psum_s.tile([P_PART, 2], f32, tag="zg_ps")
                z_ps = zg_ps[0:1, 0:1]
                nc.tensor.matmul(
                    z_ps, lhsT=es[:], rhs=ones_col[:], start=True, stop=True
                )
                c_sb = smalls.tile([1, R], f32, tag="c_sb")
                nc.scalar.copy(out=c_sb[:], in_=c_ps[:])
                zr = smalls.tile([1, 1], f32, tag="zr")
                nc.vector.reciprocal(out=zr[:], in_=z_ps)
                cprod = smalls.tile([1, R], f32, tag="cprod")
                dq = smalls.tile([1, 1], f32, tag="dq")
                nc.vector.scalar_tensor_tensor(
                    out=cprod[:],
                    in0=c_sb[:],
                    scalar=1.0,
                    in1=w2c[:],
                    op0=mybir.AluOpType.mult,
                    op1=mybir.AluOpType.mult,
                    accum_out=dq[:],
                )
                g = smalls.tile([1, 1], f32, tag="g")
                nc.vector.tensor_mul(g[:], dq[:], zr[:])
                g_ps = zg_ps[:, 1:2]
                nc.tensor.matmul(
                    g_ps, lhsT=ones_row[:], rhs=g[:], start=True, stop=True,
                    skip_group_check=True,
                )
                g_bc = smalls.tile([P_PART, 1], f32, tag="g_bc")
                nc.scalar.copy(out=g_bc[:], in_=g_ps)
                pi_b = perb.tile([P_PART, nt], f32, tag="pi_b")
                nc.scalar.activation(
                    out=pi_b[:],
                    in_=sc_b[:, :, 1],
                    func=mybir.ActivationFunctionType.Identity,
                    bias=g_bc[:],
                    scale=1.0,
                )
                # s decomposes as (st, p, j): 16 B DRAM runs; scalar HWDGE
                # ring so the tiny packets never block the input stream.
                nc.scalar.dma_start(
                    out=out_h[b].rearrange(
                        "(st p j) -> p st j", p=P_PART, j=st_sz
                    ),
                    in_=pi_b[:],
                )

    if finalize:
        nc.finalize()
    return nc


def _get_nc():
    global _CACHED_NC
    if _CACHED_NC is None:
        _CACHED_NC = _build_nc()
    return _CACHED_NC


def run_sharded(pointer_input, W1, W2, trace=False, trace_kwargs=None, nc=None):
    """Run the SPMD kernel; returns (full_output [1,B,S], BassKernelResults)."""
    from concourse.bass_utils import run_bass_kernel_spmd

    if nc is None:
        nc = _get_nc()
    pointer_input = np.ascontiguousarray(pointer_input, dtype=np.float32)
    W1 = np.ascontiguousarray(W1, dtype=np.float32)
    W2 = np.ascontiguousarray(W2, dtype=np.float32)
    ident = np.eye(P_PART, dtype=np.float32)
    in_maps = [
        {
            "p": pointer_input[i * B_LOC : (i + 1) * B_LOC],
            "w1": W1,
            "w2": W2,
            "ident": ident,
        }
        for i in range(N_CORES)
    ]
    kw = dict(trace_kwargs or {})
    res = run_bass_kernel_spmd(
        nc, in_maps, list(range(N_CORES)), trace=trace, **kw
    )
    out = np.concatenate([res.results[i]["out"] for i in range(N_CORES)], axis=0)
    return out[None].astype(np.float32), res


def kernel(pointer_input, h_t, W1, W2):
    # h_t only shifts scores by a per-batch constant, which softmax cancels;
    # it does not affect the output.
    out, _ = run_sharded(pointer_input, W1, W2, trace=False)
    return out


# revision 18
# speedup vs baseline: 1.6121x; 1.0749x over previous
"""Pointer-attention kernel for Trainium2 (8 NeuronCores, data-parallel over batch).

Computes, for P = pointer_input [B, S, R], weights W1/W2 [2R]:
    scores = P @ W1[:R] + (h @ W1[R:])[:, None]      # h-term is constant over S
    a      = softmax(scores, axis=S)                 #   -> cancels in softmax
    c      = einsum('bsr,bs->br', P, a)
    pi     = P @ W2[:R] + (c @ W2[R:])[:, None]

Math used here (exact):
    s1[b,s]  = P[b,s,:] . w1p          (w1p = W1[:R])
    E        = exp(s1)                 (softmax shift cancels; inputs are O(1))
    Z[b]     = sum_s E[b,s]
    craw[b,:]= sum_s E[b,s] * P[b,s,:]
    g[b]     = (craw[b,:] . w2c) / Z[b]            (w2c = W2[R:])
    pi[b,s]  = P[b,s,:] . w2p + g[b]               (w2p = W2[:R])

so h_t and W1[R:] never affect the output. One single pass over P.

Measured-cost engine split (per 128x512 s-tile; 8 batches x 16 tiles per core):
  - input DMA on HWDGE (nc.sync): 2 MiB p-major super-tiles in two 1 MiB
    halves -> 16 KB contiguous descriptors; GpSimd/Pool does NO compute or
    DMA (measured: concurrent Q7 tensor ops throttle DVE ~7x).
  - craw on TensorE: lhsT = exp column (bf16), rhs = the *high half-word
    view* of the fp32 tile (stride-2 bf16 bitcast) - P is never cast.
  - matvec tasks (s1+pw2 per tile) run on two engines:
      DVE path: fused scalar_tensor_tensor fp32 (~0.68us/task)
      PE path (tile pairs): 8 PE transposes (fp32, via identity) -> ACT
        evict -> 4 accumulating fp32 matmuls with [w1|w2] stationary
        (both scores at once) -> tiny PE back-transpose -> ACT evict into
        the score columns. Offloads 4 tasks/pair to the idle TensorE.
  - scores kept interleaved in sc_b [128, nt, 2] (s1, pw2) so the PE path
    lands both columns with one copy; exp/pi read strided views.
Per-b epilogue: Z via ones-matmul, dq = craw.w2c (fused DVE op), g = dq/Z,
broadcast via ones-matmul, pi = pw2 + g on ScalarE, DMA out on the scalar
HWDGE ring (tiny packets never block the input stream).
"""

import numpy as np

B, S, R = 64, 2048, 512
N_CORES = 8
B_LOC = B // N_CORES          # 8 batches per core
P_PART = 128                  # partitions per s-tile
NT = S // P_PART              # 16 s-tiles per batch

_CACHED_NC = None

# build-time strategy knobs (tuned from traces)
CFG = dict(
    st=8,            # s-tiles per DMA super-tile (8 -> 2 MiB transfers)
    pe_pairs=26,     # of 64 tile-pairs per core, how many via TensorE
    dma_eng="sync",  # sync | gpsimd
)


def _build_nc(cfg=None, b_loc=B_LOC, nt=NT, finalize=True):
    import concourse.bacc as bacc
    import concourse.bass as bass
    import concourse.mybir as mybir
    import concourse.tile as tile

    cfg = dict(CFG, **(cfg or {}))
    f32 = mybir.dt.float32
    bf16 = mybir.dt.bfloat16
    st_sz = cfg["st"]
    s_loc = nt * P_PART
    assert nt % st_sz == 0
    nst = nt // st_sz
    n_pairs_st = st_sz // 2            # tile-pairs per super-tile
    nc = bacc.Bacc(None, target_bir_lowering=False, debug=True)

    p_h = nc.declare_dram_parameter("p", [b_loc, s_loc, R], f32, isOutput=False)
    w1_h = nc.declare_dram_parameter("w1", [2 * R], f32, isOutput=False)
    w2_h = nc.declare_dram_parameter("w2", [2 * R], f32, isOutput=False)
    id_h = nc.declare_dram_parameter("ident", [P_PART, P_PART], f32, isOutput=False)
    out_h = nc.declare_dram_parameter("out", [b_loc, s_loc], f32, isOutput=True)

    def bcast_ap(src_ap, parts):
        # replicate a 1-D DRAM slice across `parts` partitions
        return bass.AP(
            tensor=src_ap.tensor,
            offset=src_ap.offset,
            ap=[[0, parts]] + [list(d) for d in src_ap.ap],
        )

    with tile.TileContext(nc) as tc:
        dma_eng = nc.sync if cfg["dma_eng"] == "sync" else nc.gpsimd
        with (
            tc.tile_pool(name="consts", bufs=1) as consts,
            tc.tile_pool(name="ptiles", bufs=5) as ptiles,
            tc.tile_pool(name="scratch", bufs=6) as scratch,
            tc.tile_pool(name="ptsb", bufs=3) as ptsb,
            tc.tile_pool(name="perb", bufs=3) as perb,
            tc.tile_pool(name="smalls", bufs=4) as smalls,
            tc.tile_pool(name="psum_c", bufs=1, space="PSUM") as psum_c,
            tc.tile_pool(name="psum_t", bufs=3, space="PSUM") as psum_t,
            tc.tile_pool(name="psum_m", bufs=2, space="PSUM") as psum_m,
            tc.tile_pool(name="psum_b", bufs=1, space="PSUM") as psum_b,
            tc.tile_pool(name="psum_s", bufs=1, space="PSUM") as psum_s,
        ):
            # ---- constants (all tiny row loads + on-chip broadcast) ----
            ident = consts.tile([P_PART, P_PART], f32)
            nc.sync.dma_start(out=ident[:], in_=id_h[:, :])
            identb = consts.tile([P_PART, P_PART], bf16)
            nc.gpsimd.dma_start(out=identb[:], in_=id_h[:, :])
            w1row = consts.tile([1, R], f32)
            nc.sync.dma_start(out=w1row[:], in_=bcast_ap(w1_h[0:R], 1))
            w2row = consts.tile([1, R], f32)
            nc.sync.dma_start(out=w2row[:], in_=bcast_ap(w2_h[0:R], 1))
            w2c = consts.tile([1, R], f32)
            nc.sync.dma_start(out=w2c[:], in_=bcast_ap(w2_h[R : 2 * R], 1))
            # w12t[rl, rc*2+k]: weight value w_k[rc*128+rl] down the
            # partitions (stationary for the PE matvec; one-time load)
            w12t = consts.tile([P_PART, 8], bf16)
            nc.gpsimd.dma_start(
                out=w12t[:, 0::2],
                in_=w1_h[0:R].rearrange("(rc rl) -> rl rc", rl=P_PART),
            )
            nc.gpsimd.dma_start(
                out=w12t[:, 1::2],
                in_=w2_h[0:R].rearrange("(rc rl) -> rl rc", rl=P_PART),
            )
            ones_col = consts.tile([P_PART, 1], f32)
            nc.vector.memset(ones_col[:], 1.0)
            ones_row = consts.tile([1, P_PART], f32)
            nc.vector.memset(ones_row[:], 1.0)
            # partition-broadcast weights for the DVE stt path
            w1p = consts.tile([P_PART, R], f32)
            w2p = consts.tile([P_PART, R], f32)
            for wrow, wp in ((w1row, w1p), (w2row, w2p)):
                w_ps = psum_t.tile([P_PART, R], f32, tag="pt_ps")
                nc.tensor.matmul(
                    w_ps[:], lhsT=ones_row[:], rhs=wrow[:], start=True, stop=True
                )
                nc.scalar.copy(out=wp[:], in_=w_ps[:])

            for b in range(b_loc):
                c_ps = psum_c.tile([1, R], f32, tag="c_ps")
                # interleaved scores: sc_b[:, t, 0] = s1, sc_b[:, t, 1] = pw2
                sc_b = perb.tile([P_PART, nt, 2], f32, tag="sc_b")
                e_bf = perb.tile([P_PART, nt], bf16, tag="e_bf")

                for sti in range(nst):
                    pt = ptiles.tile([P_PART, st_sz * R], f32, tag="pt")
                    src = p_h[b, sti * st_sz * P_PART : (sti + 1) * st_sz * P_PART, :]
                    # p-major: partition p holds st_sz consecutive DRAM rows
                    # -> 16 KB contiguous runs; two 1 MiB halves so consumers
                    # of the first half start before the second half lands.
                    src2 = src.rearrange("(p t) r -> p (t r)", p=P_PART)
                    hw = st_sz * R // 2
                    dma_eng.dma_start(out=pt[:, :hw], in_=src2[:, :hw])
                    dma_eng.dma_start(out=pt[:, hw:], in_=src2[:, hw:])

                    for pr in range(n_pairs_st):
                        j0 = 2 * pr
                        gp = (b * nst + sti) * n_pairs_st + pr
                        n_all = b_loc * nst * n_pairs_st
                        if (gp * cfg["pe_pairs"]) % n_all < cfg["pe_pairs"]:
                            # ---- PE path: transpose pair, matvec, back ----
                            pt_sb = ptsb.tile([P_PART, 2 * R], bf16, tag="pt_sb")
                            for u in range(2):
                                pt_ps = psum_t.tile([P_PART, R], bf16, tag="pt_ps")
                                for rc in range(4):
                                    chunk = pt[
                                        :,
                                        (j0 + u) * R + rc * P_PART : (j0 + u) * R + (rc + 1) * P_PART,
                                    ].bitcast(bf16)[:, 1::2]
                                    nc.tensor.transpose(
                                        pt_ps[:, rc * P_PART : (rc + 1) * P_PART],
                                        chunk,
                                        identb[:],
                                    )
                                nc.scalar.copy(
                                    out=pt_sb[:, u * R : (u + 1) * R], in_=pt_ps[:]
                                )
                            ptv = pt_sb[:].rearrange(
                                "p (u rc s) -> p rc u s", u=2, rc=4
                            )
                            mv_ps = psum_m.tile([2, 2 * P_PART], f32, tag="mv_ps")
                            for rc in range(4):
                                nc.tensor.matmul(
                                    mv_ps[:],
                                    lhsT=w12t[:, rc * 2 : rc * 2 + 2],
                                    rhs=ptv[:, rc, :, :],
                                    start=(rc == 0),
                                    stop=(rc == 3),
                                )
                            mv_sb = smalls.tile([2, 2 * P_PART], f32, tag="mv_sb")
                            nc.scalar.copy(out=mv_sb[:], in_=mv_ps[:])
                            for u in range(2):
                                t = sti * st_sz + j0 + u
                                bt_ps = psum_b.tile([P_PART, 2], f32, tag="bt_ps")
                                nc.tensor.transpose(
                                    bt_ps[:],
                                    mv_sb[:, u * P_PART : (u + 1) * P_PART],
                                    ident[0:2, 0:2],
                                )
                                nc.scalar.copy(
                                    out=sc_b[:, t, :], in_=bt_ps[:]
                                )
                        else:
                            # ---- DVE path: fused stt per tile & weight ----
                            for u in range(2):
                                t = sti * st_sz + j0 + u
                                for w, wp in ((0, w1p), (1, w2p)):
                                    prod = scratch.tile([P_PART, R], f32, tag="prod")
                                    nc.vector.scalar_tensor_tensor(
                                        out=prod[:],
                                        in0=pt[:, (j0 + u) * R : (j0 + u + 1) * R],
                                        scalar=1.0,
                                        in1=wp[:],
                                        op0=mybir.AluOpType.mult,
                                        op1=mybir.AluOpType.mult,
                                        accum_out=sc_b[:, t, w : w + 1],
                                    )

                    # ---- exp of this super-tile's score columns (bf16) ----
                    nc.scalar.activation(
                        out=e_bf[:, sti * st_sz : (sti + 1) * st_sz],
                        in_=sc_b[:, sti * st_sz : (sti + 1) * st_sz, 0],
                        func=mybir.ActivationFunctionType.Exp,
                    )
                    # ---- craw accumulation on TensorE (hi-half view) ----
                    for j in range(st_sz):
                        t = sti * st_sz + j
                        rhs_hi = pt[:, j * R : (j + 1) * R].bitcast(bf16)[:, 1::2]
                        nc.tensor.matmul(
                            c_ps[:],
                            lhsT=e_bf[:, t : t + 1],
                            rhs=rhs_hi,
                            start=(t == 0),
                            stop=(t == nt - 1),
                        )

                # ---- per-batch epilogue (all tiny, fp32) ----
                es = smalls.tile([P_PART, 1], f32, tag="es")
                nc.vector.reduce_sum(es[:], e_bf[:], axis=mybir.AxisListType.X)
                zg_ps = psum_s.tile([P_PART, 2], f32, tag="zg_ps")
                z_ps = zg_ps[0:1, 0:1]
                nc.tensor.matmul(
                    z_ps, lhsT=es[:], rhs=ones_col[:], start=True, stop=True
                )
                c_sb = smalls.tile([1, R], f32, tag="c_sb")
                nc.scalar.copy(out=c_sb[:], in_=c_ps[:])
                zr = smalls.tile([1, 1], f32, tag="zr")
                nc.vector.reciprocal(out=zr[:], in_=z_ps)
                cprod = smalls.tile([1, R], f32, tag="cprod")
                dq = smalls.tile([1, 1], f32, tag="dq")
                nc.vector.scalar_tensor_tensor(
                    out=cprod[:],
                    in0=c_sb[:],
                    scalar=1.0,
                    in1=w2c[:],
                    op0=mybir.AluOpType.mult,
                    op1=mybir.AluOpType.mult,
                    accum_out=dq[:],
                )
                g = smalls.tile([1, 1], f32, tag="g")
                nc.vector.tensor_mul(g[:], dq[:], zr[:])
                g_ps = zg_ps[:, 1:2]
                nc.tensor.matmul(
                    g_ps, lhsT=ones_row[:], rhs=g[:], start=True, stop=True,
                    skip_group_check=True,
                )
                g_bc = smalls.tile([P_PART, 1], f32, tag="g_bc")
                nc.scalar.copy(out=g_bc[:], in_=g_ps)
                pi_b = perb.tile([P_PART, nt], f32, tag="pi_b")
                nc.scalar.activation(
                    out=pi_b[:],
                    in_=sc_b[:, :, 1],
                    func=mybir.ActivationFunctionType.Identity,
                    bias=g_bc[:],
                    scale=1.0,
                )
                # s decomposes as (st, p, j): 16 B DRAM runs; scalar HWDGE
                # ring so the tiny packets never block the input stream.
                nc.scalar.dma_start(
                    out=out_h[b].rearrange(
                        "(st p j) -> p st j", p=P_PART, j=st_sz
                    ),
                    in_=pi_b[:],
                )

    if finalize:
        nc.finalize()
    return nc


def _get_nc():
    global _CACHED_NC
    if _CACHED_NC is None:
        _CACHED_NC = _build_nc()
    return _CACHED_NC


def run_sharded(pointer_input, W1, W2, trace=False, trace_kwargs=None, nc=None):
    """Run the SPMD kernel; returns (full_output [1,B,S], BassKernelResults)."""
    from concourse.bass_utils import run_bass_kernel_spmd

    if nc is None:
        nc = _get_nc()
    pointer_input = np.ascontiguousarray(pointer_input, dtype=np.float32)
    W1 = np.ascontiguousarray(W1, dtype=np.float32)
    W2 = np.ascontiguousarray(W2, dtype=np.float32)
    ident = np.eye(P_PART, dtype=np.float32)
    in_maps = [
        {
            "p": pointer_input[i * B_LOC : (i + 1) * B_LOC],
            "w1": W1,
            "w2": W2,
            "ident": ident,
        }
        for i in range(N_CORES)
    ]
    kw = dict(trace_kwargs or {})
    res = run_bass_kernel_spmd(
        nc, in_maps, list(range(N_CORES)), trace=trace, **kw
    )
    out = np.concatenate([res.results[i]["out"] for i in range(N_CORES)], axis=0)
    return out[None].astype(np.float32), res


def kernel(pointer_input, h_t, W1, W2):
    # h_t only shifts scores by a per-batch constant, which softmax cancels;
    # it does not affect the output.
    out, _ = run_sharded(pointer_input, W1, W2, trace=False)
    return out


# revision 19
# speedup vs baseline: 1.6655x; 1.0331x over previous
"""Pointer-attention kernel for Trainium2 (8 NeuronCores, data-parallel over batch).

Computes, for P = pointer_input [B, S, R], weights W1/W2 [2R]:
    scores = P @ W1[:R] + (h @ W1[R:])[:, None]      # h-term is constant over S
    a      = softmax(scores, axis=S)                 #   -> cancels in softmax
    c      = einsum('bsr,bs->br', P, a)
    pi     = P @ W2[:R] + (c @ W2[R:])[:, None]

Math used here (exact):
    s1[b,s]  = P[b,s,:] . w1p          (w1p = W1[:R])
    E        = exp(s1)                 (softmax shift cancels; inputs are O(1))
    Z[b]     = sum_s E[b,s]
    craw[b,:]= sum_s E[b,s] * P[b,s,:]
    g[b]     = (craw[b,:] . w2c) / Z[b]            (w2c = W2[R:])
    pi[b,s]  = P[b,s,:] . w2p + g[b]               (w2p = W2[:R])

so h_t and W1[R:] never affect the output. One single pass over P.

Measured-cost engine split (per 128x512 s-tile; 8 batches x 16 tiles per core):
  - input DMA on HWDGE (nc.sync): 2 MiB p-major super-tiles in two 1 MiB
    halves -> 16 KB contiguous descriptors; GpSimd/Pool does NO compute or
    DMA (measured: concurrent Q7 tensor ops throttle DVE ~7x).
  - craw on TensorE: lhsT = exp column (bf16), rhs = the *high half-word
    view* of the fp32 tile (stride-2 bf16 bitcast) - P is never cast.
  - matvec tasks (s1+pw2 per tile) run on two engines:
      DVE path: fused scalar_tensor_tensor fp32 (~0.68us/task)
      PE path (tile pairs): 8 PE transposes (fp32, via identity) -> ACT
        evict -> 4 accumulating fp32 matmuls with [w1|w2] stationary
        (both scores at once) -> tiny PE back-transpose -> ACT evict into
        the score columns. Offloads 4 tasks/pair to the idle TensorE.
  - scores kept interleaved in sc_b [128, nt, 2] (s1, pw2) so the PE path
    lands both columns with one copy; exp/pi read strided views.
Per-b epilogue: Z via ones-matmul, dq = craw.w2c (fused DVE op), g = dq/Z,
broadcast via ones-matmul, pi = pw2 + g on ScalarE, DMA out on the scalar
HWDGE ring (tiny packets never block the input stream).
"""

import numpy as np

B, S, R = 64, 2048, 512
N_CORES = 8
B_LOC = B // N_CORES          # 8 batches per core
P_PART = 128                  # partitions per s-tile
NT = S // P_PART              # 16 s-tiles per batch

_CACHED_NC = None

# build-time strategy knobs (tuned from traces)
CFG = dict(
    st=8,            # s-tiles per DMA super-tile (8 -> 2 MiB transfers)
    pe_pairs=25,     # of 64 tile-pairs per core, how many via TensorE
    dma_eng="sync",  # sync | gpsimd
)


def _build_nc(cfg=None, b_loc=B_LOC, nt=NT, finalize=True):
    import concourse.bacc as bacc
    import concourse.bass as bass
    import concourse.mybir as mybir
    import concourse.tile as tile

    cfg = dict(CFG, **(cfg or {}))
    f32 = mybir.dt.float32
    bf16 = mybir.dt.bfloat16
    st_sz = cfg["st"]
    s_loc = nt * P_PART
    assert nt % st_sz == 0
    nst = nt // st_sz
    n_pairs_st = st_sz // 2            # tile-pairs per super-tile
    nc = bacc.Bacc(None, target_bir_lowering=False, debug=True)

    p_h = nc.declare_dram_parameter("p", [b_loc, s_loc, R], f32, isOutput=False)
    w1_h = nc.declare_dram_parameter("w1", [2 * R], f32, isOutput=False)
    w2_h = nc.declare_dram_parameter("w2", [2 * R], f32, isOutput=False)
    id_h = nc.declare_dram_parameter("ident", [P_PART, P_PART], f32, isOutput=False)
    out_h = nc.declare_dram_parameter("out", [b_loc, s_loc], f32, isOutput=True)

    def bcast_ap(src_ap, parts):
        # replicate a 1-D DRAM slice across `parts` partitions
        return bass.AP(
            tensor=src_ap.tensor,
            offset=src_ap.offset,
            ap=[[0, parts]] + [list(d) for d in src_ap.ap],
        )

    with tile.TileContext(nc) as tc:
        dma_eng = nc.sync if cfg["dma_eng"] == "sync" else nc.gpsimd
        with (
            tc.tile_pool(name="consts", bufs=1) as consts,
            tc.tile_pool(name="ptiles", bufs=6) as ptiles,
            tc.tile_pool(name="scratch", bufs=6) as scratch,
            tc.tile_pool(name="ptsb", bufs=3) as ptsb,
            tc.tile_pool(name="perb", bufs=3) as perb,
            tc.tile_pool(name="smalls", bufs=4) as smalls,
            tc.tile_pool(name="psum_c", bufs=1, space="PSUM") as psum_c,
            tc.tile_pool(name="psum_t", bufs=3, space="PSUM") as psum_t,
            tc.tile_pool(name="psum_m", bufs=2, space="PSUM") as psum_m,
            tc.tile_pool(name="psum_b", bufs=1, space="PSUM") as psum_b,
            tc.tile_pool(name="psum_s", bufs=1, space="PSUM") as psum_s,
        ):
            # ---- constants (all tiny row loads + on-chip broadcast) ----
            ident = consts.tile([P_PART, P_PART], f32)
            nc.sync.dma_start(out=ident[:], in_=id_h[:, :])
            identb = consts.tile([P_PART, P_PART], bf16)
            nc.gpsimd.dma_start(out=identb[:], in_=id_h[:, :])
            w1row = consts.tile([1, R], f32)
            nc.sync.dma_start(out=w1row[:], in_=bcast_ap(w1_h[0:R], 1))
            w2row = consts.tile([1, R], f32)
            nc.sync.dma_start(out=w2row[:], in_=bcast_ap(w2_h[0:R], 1))
            w2c = consts.tile([1, R], f32)
            nc.sync.dma_start(out=w2c[:], in_=bcast_ap(w2_h[R : 2 * R], 1))
            # w12t[rl, rc*2+k]: weight value w_k[rc*128+rl] down the
            # partitions (stationary for the PE matvec; one-time load)
            w12t = consts.tile([P_PART, 8], bf16)
            nc.gpsimd.dma_start(
                out=w12t[:, 0::2],
                in_=w1_h[0:R].rearrange("(rc rl) -> rl rc", rl=P_PART),
            )
            nc.gpsimd.dma_start(
                out=w12t[:, 1::2],
                in_=w2_h[0:R].rearrange("(rc rl) -> rl rc", rl=P_PART),
            )
            ones_col = consts.tile([P_PART, 1], f32)
            nc.vector.memset(ones_col[:], 1.0)
            ones_row = consts.tile([1, P_PART], f32)
            nc.vector.memset(ones_row[:], 1.0)
            # partition-broadcast weights for the DVE stt path
            w1p = consts.tile([P_PART, R], f32)
            w2p = consts.tile([P_PART, R], f32)
            for wrow, wp in ((w1row, w1p), (w2row, w2p)):
                w_ps = psum_t.tile([P_PART, R], f32, tag="pt_ps")
                nc.tensor.matmul(
                    w_ps[:], lhsT=ones_row[:], rhs=wrow[:], start=True, stop=True
                )
                nc.scalar.copy(out=wp[:], in_=w_ps[:])

            for b in range(b_loc):
                c_ps = psum_c.tile([1, R], f32, tag="c_ps")
                # interleaved scores: sc_b[:, t, 0] = s1, sc_b[:, t, 1] = pw2
                sc_b = perb.tile([P_PART, nt, 2], f32, tag="sc_b")
                e_bf = perb.tile([P_PART, nt], bf16, tag="e_bf")

                for sti in range(nst):
                    pt = ptiles.tile([P_PART, st_sz * R], f32, tag="pt")
                    src = p_h[b, sti * st_sz * P_PART : (sti + 1) * st_sz * P_PART, :]
                    # p-major: partition p holds st_sz consecutive DRAM rows
                    # -> 16 KB contiguous runs; two 1 MiB halves so consumers
                    # of the first half start before the second half lands.
                    src2 = src.rearrange("(p t) r -> p (t r)", p=P_PART)
                    hw = st_sz * R // 2
                    dma_eng.dma_start(out=pt[:, :hw], in_=src2[:, :hw])
                    dma_eng.dma_start(out=pt[:, hw:], in_=src2[:, hw:])

                    for pr in range(n_pairs_st):
                        j0 = 2 * pr
                        gp = (b * nst + sti) * n_pairs_st + pr
                        n_all = b_loc * nst * n_pairs_st
                        if (gp * cfg["pe_pairs"]) % n_all < cfg["pe_pairs"]:
                            # ---- PE path: transpose pair, matvec, back ----
                            pt_sb = ptsb.tile([P_PART, 2 * R], bf16, tag="pt_sb")
                            for u in range(2):
                                pt_ps = psum_t.tile([P_PART, R], bf16, tag="pt_ps")
                                for rc in range(4):
                                    chunk = pt[
                                        :,
                                        (j0 + u) * R + rc * P_PART : (j0 + u) * R + (rc + 1) * P_PART,
                                    ].bitcast(bf16)[:, 1::2]
                                    nc.tensor.transpose(
                                        pt_ps[:, rc * P_PART : (rc + 1) * P_PART],
                                        chunk,
                                        identb[:],
                                    )
                                nc.scalar.copy(
                                    out=pt_sb[:, u * R : (u + 1) * R], in_=pt_ps[:]
                                )
                            ptv = pt_sb[:].rearrange(
                                "p (u rc s) -> p rc u s", u=2, rc=4
                            )
                            mv_ps = psum_m.tile([2, 2 * P_PART], f32, tag="mv_ps")
                            for rc in range(4):
                                nc.tensor.matmul(
                                    mv_ps[:],
                                    lhsT=w12t[:, rc * 2 : rc * 2 + 2],
                                    rhs=ptv[:, rc, :, :],
                                    start=(rc == 0),
                                    stop=(rc == 3),
                                )
                            mv_sb = smalls.tile([2, 2 * P_PART], f32, tag="mv_sb")
                            nc.scalar.copy(out=mv_sb[:], in_=mv_ps[:])
                            for u in range(2):
                                t = sti * st_sz + j0 + u
                                bt_ps = psum_b.tile([P_PART, 2], f32, tag="bt_ps")
                                nc.tensor.transpose(
                                    bt_ps[:],
                                    mv_sb[:, u * P_PART : (u + 1) * P_PART],
                                    ident[0:2, 0:2],
                                )
                                nc.scalar.copy(
                                    out=sc_b[:, t, :], in_=bt_ps[:]
                                )
                        else:
                            # ---- DVE path: fused stt per tile & weight ----
                            for u in range(2):
                                t = sti * st_sz + j0 + u
                                for w, wp in ((0, w1p), (1, w2p)):
                                    prod = scratch.tile([P_PART, R], f32, tag="prod")
                                    nc.vector.scalar_tensor_tensor(
                                        out=prod[:],
                                        in0=pt[:, (j0 + u) * R : (j0 + u + 1) * R],
                                        scalar=1.0,
                                        in1=wp[:],
                                        op0=mybir.AluOpType.mult,
                                        op1=mybir.AluOpType.mult,
                                        accum_out=sc_b[:, t, w : w + 1],
                                    )

                    # ---- exp of this super-tile's score columns (bf16) ----
                    nc.scalar.activation(
                        out=e_bf[:, sti * st_sz : (sti + 1) * st_sz],
                        in_=sc_b[:, sti * st_sz : (sti + 1) * st_sz, 0],
                        func=mybir.ActivationFunctionType.Exp,
                    )
                    # ---- craw accumulation on TensorE (hi-half view) ----
                    for j in range(st_sz):
                        t = sti * st_sz + j
                        rhs_hi = pt[:, j * R : (j + 1) * R].bitcast(bf16)[:, 1::2]
                        nc.tensor.matmul(
                            c_ps[:],
                            lhsT=e_bf[:, t : t + 1],
                            rhs=rhs_hi,
                            start=(t == 0),
                            stop=(t == nt - 1),
                        )

                # ---- per-batch epilogue (all tiny, fp32) ----
                es = smalls.tile([P_PART, 1], f32, tag="es")
                nc.vector.reduce_sum(es[:], e_bf[:], axis=mybir.AxisListType.X)
                zg_ps = psum_s.tile([P_PART, 2], f32, tag="zg_ps")
                z_ps = zg_ps[0:1, 0:1]
                nc.tensor.matmul(
                    z_ps, lhsT=es[:], rhs=ones_col[:], start=True, stop=True
                )
                c_sb = smalls.tile([1, R], f32, tag="c_sb")
                nc.scalar.copy(out=c_sb[:], in_=c_ps[:])
                zr = smalls.tile([1, 1], f32, tag="zr")
                nc.vector.reciprocal(out=zr[:], in_=z_ps)
                cprod = smalls.tile([1, R], f32, tag="cprod")
                dq = smalls.tile([1, 1], f32, tag="dq")
                nc.vector.scalar_tensor_tensor(
                    out=cprod[:],
                    in0=c_sb[:],
                    scalar=1.0,
                    in1=w2c[:],
                    op0=mybir.AluOpType.mult,
                    op1=mybir.AluOpType.mult,
                    accum_out=dq[:],
                )
                g = smalls.tile([1, 1], f32, tag="g")
                nc.vector.tensor_mul(g[:], dq[:], zr[:])
                g_ps = zg_ps[:, 1:2]
                nc.tensor.matmul(
                    g_ps, lhsT=ones_row[:], rhs=g[:], start=True, stop=True,
                    skip_group_check=True,
                )
                g_bc = smalls.tile([P_PART, 1], f32, tag="g_bc")
                nc.scalar.copy(out=g_bc[:], in_=g_ps)
                pi_b = perb.tile([P_PART, nt], f32, tag="pi_b")
                nc.scalar.activation(
                    out=pi_b[:],
                    in_=sc_b[:, :, 1],
                    func=mybir.ActivationFunctionType.Identity,
                    bias=g_bc[:],
                    scale=1.0,
                )
                # s decomposes as (st, p, j): 16 B DRAM runs; scalar HWDGE
                # ring so the tiny packets never block the input stream.
                nc.scalar.dma_start(
                    out=out_h[b].rearrange(
                        "(st p j) -> p st j", p=P_PART, j=st_sz
                    ),
                    in_=pi_b[:],
                )

    if finalize:
        nc.finalize()
    return nc


def _get_nc():
    global _CACHED_NC
    if _CACHED_NC is None:
        _CACHED_NC = _build_nc()
    return _CACHED_NC


def run_sharded(pointer_input, W1, W2, trace=False, trace_kwargs=None, nc=None):
    """Run the SPMD kernel; returns (full_output [1,B,S], BassKernelResults)."""
    from concourse.bass_utils import run_bass_kernel_spmd

    if nc is None:
        nc = _get_nc()
    pointer_input = np.ascontiguousarray(pointer_input, dtype=np.float32)
    W1 = np.ascontiguousarray(W1, dtype=np.float32)
    W2 = np.ascontiguousarray(W2, dtype=np.float32)
    ident = np.eye(P_PART, dtype=np.float32)
    in_maps = [
        {
            "p": pointer_input[i * B_LOC : (i + 1) * B_LOC],
            "w1": W1,
            "w2": W2,
            "ident": ident,
        }
        for i in range(N_CORES)
    ]
    kw = dict(trace_kwargs or {})
    res = run_bass_kernel_spmd(
        nc, in_maps, list(range(N_CORES)), trace=trace, **kw
    )
    out = np.concatenate([res.results[i]["out"] for i in range(N_CORES)], axis=0)
    return out[None].astype(np.float32), res


def kernel(pointer_input, h_t, W1, W2):
    # h_t only shifts scores by a per-batch constant, which softmax cancels;
    # it does not affect the output.
    out, _ = run_sharded(pointer_input, W1, W2, trace=False)
    return out
